# revision 13
# baseline (speedup 1.0000x reference)
"""DCCA 2D loss kernel for 8 Trainium2 NeuronCores (Bass/Tile).

Strategy (data-parallel over the m = B*C = 2048 sample axis):
  - Each core gets 256 samples of both views, host-stacked into one tensor
    xx [2, NS, 64, 128].  Per chunk (16..64 samples) ONE SWDGE dma_start
    loads both views with f32->bf16 cast; partition = (sample-in-block)*8
    + n//8 so every descriptor reads a contiguous 4KB row-run.  Per
    16-sample block the PE transposes eight [128,128] slices (matmul
    against the identity), a vector/scalar copy assembles per-sample fused
    tiles T_m = [H1_m^T | H2_m^T] (n lands in a fixed even permutation,
    under which the final scalar is invariant), and the fused Gram
    G += T_m^T T_m accumulates in PSUM.  The ridge enters as the first
    PSUM matmul: weye^T weye = (R/(8*C1)) * I, so the AllReduce output is
    exactly S/C1 -- and the trace-normalized epilogue below is exactly
    invariant to the global scale C1, which therefore never appears.
  - One 64KB AllReduce(add) of the partial Gram (warmed up by a tiny
    AllReduce issued at t=0 so the ncfw control plane is hot).
  - Replicated epilogue without eigh: with  At = blockdiag(S11, S22)/C1,
    t = 128/tr(At), the closed-form first Newton-Schulz iterate
    X1 = t*(2I - t*At)  already inverts At to ~1e-4 (eigenvalues of t*At
    lie in [0.99, 1.01] for covariance data), optionally refined by
    BASS_NEWTON_ITERS more iterations (default 0).  Then
    corr^2 = sum( (X1 St)[0:64,64:128] * (St X1)[0:64,64:128] )  and
    out = -sqrt(corr^2).
"""

import math
import os

import numpy as np

# ---------------------------------------------------------------- constants
B, C, N, K = 32, 64, 64, 128
M = B * C                    # 2048 samples
NC = 8                       # cores
NS = M // NC                 # 256 samples per core
# Samples per DMA chunk. First chunk small so the PE starts within ~2us;
# tail tapered so the PE isn't left with a big block after the last byte.
CHUNKS = (16, 48, 64, 64, 32, 16, 16)
assert sum(CHUNKS) == NS
R_RIDGE = 1e-4
C1 = (1.0 - 1.0 / M) ** 2 / (M * (M - 1))
W_RIDGE = math.sqrt(R_RIDGE / (NC * C1))  # per-core PSUM ridge weight

NEWTON_ITERS = int(os.environ.get("BASS_NEWTON_ITERS", "0"))
G = 16                       # samples per partition-block
NR = 8                       # consecutive n-rows per descriptor (4KB)
NU = N // NR

_CACHE = {}


def _build():
    import contextlib

    import concourse.bass as bass
    import concourse.mybir as mybir
    import concourse.tile as tile
    from concourse import bacc

    f32 = mybir.dt.float32
    bf16 = mybir.dt.bfloat16

    nc = bacc.Bacc(
        "TRN2",
        target_bir_lowering=False,
        debug=False,
        enable_asserts=False,
        num_devices=NC,
    )

    xx = nc.dram_tensor(
        "xx", [NS // G, 2, G, N, K], f32, kind="ExternalInput"
    ).ap()
    ident_d = nc.dram_tensor("ident", [128, 128], bf16, kind="ExternalInput").ap()
    weye_d = nc.dram_tensor("weye", [128, 128], bf16, kind="ExternalInput").ap()
    eye2_d = nc.dram_tensor("eye2", [128, 128], f32, kind="ExternalInput").ap()
    maskd_d = nc.dram_tensor("maskd", [128, 128], f32, kind="ExternalInput").ap()
    ones_d = nc.dram_tensor("onesf", [128, 128], f32, kind="ExternalInput").ap()
    out_d = nc.dram_tensor("out", [1, 1], f32, kind="ExternalOutput").ap()

    groups = [list(range(NC))]

    with tile.TileContext(nc) as tc:
        with contextlib.ExitStack() as ctx:
            cpool = ctx.enter_context(tc.tile_pool(name="consts", bufs=1))
            spool = ctx.enter_context(tc.tile_pool(name="work", bufs=2))
            dpool = ctx.enter_context(tc.tile_pool(name="dram", bufs=1, space="DRAM"))

            # Dependency-free tiny collective issued before everything else:
            # wakes the ncfw control plane (first collective on this runtime
            # costs ~25us) so the real AllReduce later is warm.
            win = dpool.tile([1, 16], f32)
            wout = dpool.tile([1, 16], f32)
            wsb = spool.tile([1, 16], f32, tag="wsb")
            nc.gpsimd.memset(wsb[:], 0.0)
            nc.gpsimd.dma_start(win[:], wsb[:])
            nc.gpsimd.collective_compute(
                "AllReduce",
                mybir.AluOpType.add,
                replica_groups=groups,
                ins=[win.opt()],
                outs=[wout.opt()],
            )

            ident = cpool.tile([128, 128], bf16)
            nc.sync.dma_start(ident[:], ident_d)
            weye = cpool.tile([128, 128], bf16)
            nc.sync.dma_start(weye[:], weye_d)
            eye2 = cpool.tile([128, 128], f32)
            nc.sync.dma_start(eye2[:], eye2_d)
            maskd = cpool.tile([128, 128], f32)
            nc.sync.dma_start(maskd[:], maskd_d)
            onesf = cpool.tile([128, 128], f32)
            nc.sync.dma_start(onesf[:], ones_d)

            # Warm the SQRT activation table while Scalar is idle, so the
            # epilogue's sqrt doesn't pay ACT_TABLE_LOAD + drain (~2.5us).
            wsq = spool.tile([1, 1], f32, tag="wsq")
            nc.scalar.sqrt(wsq[:], onesf[0:1, 0:1])

            gsb = spool.tile([128, 128], f32, tag="gsb")

            # ---------------- main loop: per-core partial fused Gram ------
            # Per chunk/SBUF tile V [128, 2, CH/G, NR*128] (bf16):
            #   V[8h+u, v, j, r*128+k] = X[v, s0+G*j+h, u*NR+r, k]
            # Per G-sample block: 8 transposes (2 views x 4 r) as regular
            # matmuls against the identity (FWL-friendly), one assembly copy
            # per view into the fused TT tile, then G Gram matmuls.
            with (
                tc.tile_pool(name="vload", bufs=3) as vpool,
                tc.tile_pool(name="ttp", bufs=3) as ttpool,
                tc.tile_pool(name="ptp", bufs=2, space="PSUM") as ptpool,
                tc.tile_pool(name="gpp", bufs=1, space="PSUM") as gpool,
            ):
                gp = gpool.tile([128, 128], f32, tag="gp")
                # ridge: weye^T weye = (R/(8*C1)) I seeds the accumulator
                nc.tensor.matmul(gp[:], weye[:], weye[:], start=True, stop=False)

                n_blocks_total = NS // G

                def emit_gram(tt8, bi):
                    last = bi == n_blocks_total - 1
                    tt8f = tt8.rearrange("p h b u -> p (h b u)")
                    for h in range(G):
                        nc.tensor.matmul(
                            gp[:],
                            tt8f[:, 128 * h : 128 * (h + 1)],
                            tt8f[:, 128 * h : 128 * (h + 1)],
                            start=False,
                            stop=last and h == G - 1,
                        )

                pending = None  # (tt, block_index) awaiting Gram matmuls
                bi = 0
                s0 = 0
                for CH in CHUNKS:
                    nj = CH // G
                    vt = vpool.tile([128, 2 * nj, NR * 128], bf16, tag="v")
                    src = xx[s0 : s0 + nj].rearrange(
                        "j v h (u r) k -> (h u) (j v) (r k)", r=NR
                    )
                    # SWDGE casts f32 -> bf16 during the transfer.
                    nc.gpsimd.dma_start(vt[:], src)
                    s0 += nj

                    for j in range(nj):
                        tt = ttpool.tile([128, G, 2, 64], bf16, tag="tt")
                        for vi in range(2):
                            # bf16 PSUM transpose tile: values are already
                            # bf16-exact, and the PSUM->SBUF copy runs at the
                            # DVE 16-bit 2x rate.
                            ptv = ptpool.tile([128, NR, 128], bf16, tag=f"pt{vi}")
                            for r in range(NR):
                                nc.tensor.transpose(
                                    ptv[:, r, :],
                                    vt[:, 2 * j + vi, r * 128 : (r + 1) * 128],
                                    ident[:],
                                )
                            nc.any.tensor_copy(
                                out=tt[:, :, vi, :].rearrange(
                                    "p h (r u) -> p h r u", r=NR
                                ),
                                in_=ptv.rearrange("p r (h u) -> p h r u", h=G),
                            )
                        # one-block software pipeline: this block's Gram
                        # matmuls are emitted after the NEXT block's
                        # transposes, so the PE never stalls on the copy.
                        if pending is not None:
                            emit_gram(*pending)
                        pending = (tt, bi)
                        bi += 1
                emit_gram(*pending)

                nc.vector.tensor_copy(gsb[:], gp[:])

            # ---------------- AllReduce + replicated epilogue -------------
            with tc.tile_pool(name="epp", bufs=1, space="PSUM") as epool:
                din = dpool.tile([128, 128], f32)
                dout = dpool.tile([NC, 128, 128], f32)
                # HWDGE on the sync queue: skips the SWDGE queue tail.
                nc.sync.dma_start(din[:], gsb[:])
                # AllGather (one ncfw phase) + on-chip tree sum is ~8us
                # faster end-to-end than ncfw AllReduce (= RS + AG).
                nc.gpsimd.collective_compute(
                    "AllGather",
                    mybir.AluOpType.bypass,
                    replica_groups=groups,
                    ins=[din.opt()],
                    outs=[dout.opt()],
                )
                gall = spool.tile([128, NC, 128], f32, tag="gall")
                nc.sync.dma_start(gall[:], dout[:].rearrange("c p k -> p c k"))
                g4 = spool.tile([128, 4, 128], f32, tag="g4")
                nc.vector.tensor_add(g4[:], gall[:, 0:4, :], gall[:, 4:8, :])
                g2 = spool.tile([128, 2, 128], f32, tag="g2")
                nc.vector.tensor_add(g2[:], g4[:, 0:2, :], g4[:, 2:4, :])
                S = spool.tile([128, 128], f32, tag="S")
                nc.vector.tensor_add(S[:], g2[:, 0, :], g2[:, 1, :])

                # A = blockdiag(S); dm = 2*diag(S) (independent -> both DVE)
                A = spool.tile([128, 128], f32, tag="A")
                nc.vector.tensor_mul(A[:], S[:], maskd[:])
                dm = spool.tile([128, 128], f32, tag="dm")
                nc.vector.tensor_mul(dm[:], S[:], eye2[:])
                dcol = spool.tile([128, 1], f32, tag="dcol")
                nc.vector.reduce_sum(dcol[:], dm[:], axis=mybir.AxisListType.X)
                trp = epool.tile([128, 1], f32, tag="trp")
                nc.tensor.matmul(trp[:], onesf[:], dcol[:], start=True, stop=True)
                rcol = spool.tile([128, 1], f32, tag="rcol")
                nc.vector.reciprocal(rcol[:], trp[:])  # 1/(2 tr A)

                # Ahat = t*A (t = 128/trA = 256*rcol); Xhat = 2I - Ahat
                ahat = spool.tile([128, 128], f32, tag="ahat")
                nc.vector.tensor_scalar(
                    ahat[:], A[:], rcol[:], 256.0,
                    op0=mybir.AluOpType.mult, op1=mybir.AluOpType.mult,
                )
                xhat = spool.tile([128, 128], f32, tag="xh")
                nc.vector.tensor_tensor(
                    xhat[:], eye2[:], ahat[:], mybir.AluOpType.subtract
                )
                for _ in range(NEWTON_ITERS):
                    bp = epool.tile([128, 128], f32, tag="bp")
                    nc.tensor.matmul(bp[:], ahat[:], xhat[:], start=True, stop=True)
                    cs = spool.tile([128, 128], f32, tag="cs")
                    nc.vector.tensor_tensor(
                        cs[:], eye2[:], bp[:], mybir.AluOpType.subtract
                    )
                    xp = epool.tile([128, 128], f32, tag="xp")
                    nc.tensor.matmul(xp[:], xhat[:], cs[:], start=True, stop=True)
                    xnew = spool.tile([128, 128], f32, tag="xh")
                    nc.vector.tensor_copy(xnew[:], xp[:])
                    xhat = xnew
                # X1 = t * Xhat  (so X1 ~= inv(blockdiag) up to the global C1)
                x1 = spool.tile([128, 128], f32, tag="x1")
                nc.vector.tensor_scalar(
                    x1[:], xhat[:], rcol[:], 256.0,
                    op0=mybir.AluOpType.mult, op1=mybir.AluOpType.mult,
                )

                # corr^2 = sum( (X1 S)[0:64,64:] * (S X1)[0:64,64:] )
                up = epool.tile([64, 64], f32, tag="up")
                nc.tensor.matmul(up[:], x1[:, 0:64], S[:, 64:128], start=True, stop=True)
                vp = epool.tile([64, 64], f32, tag="vp")
                nc.tensor.matmul(vp[:], S[:, 0:64], x1[:, 64:128], start=True, stop=True)
                vps = spool.tile([64, 64], f32, tag="vps")
                nc.vector.tensor_copy(vps[:], vp[:])
                pm = spool.tile([64, 64], f32, tag="pm")
                nc.vector.tensor_tensor(pm[:], up[:], vps[:], mybir.AluOpType.mult)
                pcol = spool.tile([64, 1], f32, tag="pcol")
                nc.vector.reduce_sum(pcol[:], pm[:], axis=mybir.AxisListType.X)
                cp = epool.tile([1, 1], f32, tag="cp")
                nc.tensor.matmul(cp[:], pcol[:], onesf[0:64, 0:1], start=True, stop=True)
                root = spool.tile([1, 1], f32, tag="root")
                nc.scalar.sqrt(root[:], cp[:])
                nc.vector.tensor_scalar_mul(root[:], root[:], -1.0)
                nc.sync.dma_start(out_d, root[:])

    nc.compile()
    return nc


def _get_nc():
    key = (NEWTON_ITERS,)
    if key not in _CACHE:
        _CACHE[key] = _build()
    return _CACHE[key]


def _const_inputs():
    import ml_dtypes

    eye = np.eye(128, dtype=np.float32)
    maskd = np.zeros((128, 128), dtype=np.float32)
    maskd[:64, :64] = np.eye(64, dtype=np.float32)
    maskd[64:, 64:] = np.eye(64, dtype=np.float32)
    return {
        "ident": eye.astype(ml_dtypes.bfloat16),
        "weye": (W_RIDGE * eye).astype(ml_dtypes.bfloat16),
        "eye2": (2.0 * eye).astype(np.float32),
        "maskd": maskd,
        "onesf": np.ones((128, 128), dtype=np.float32),
    }


def kernel(data_view1, data_view2):
    from concourse import bass_utils

    h1 = np.ascontiguousarray(data_view1, dtype=np.float32).reshape(M, N, K)
    h2 = np.ascontiguousarray(data_view2, dtype=np.float32).reshape(M, N, K)

    consts = _const_inputs()
    in_maps = []
    for c in range(NC):
        m = {
            "xx": np.stack(
                [
                    h1[c * NS : (c + 1) * NS].reshape(NS // G, G, N, K),
                    h2[c * NS : (c + 1) * NS].reshape(NS // G, G, N, K),
                ],
                axis=1,
            ),
        }
        m.update(consts)
        in_maps.append(m)

    nc = _get_nc()
    trace = os.environ.get("BASS_KERNEL_TRACE", "0") == "1"
    res = bass_utils.run_bass_kernel_spmd(
        nc, in_maps, core_ids=list(range(NC)), trace=trace
    )
    if trace:
        kernel.last_results = res
    val = np.asarray(res.results[0]["out"]).reshape(())
    return val.astype(np.float32)


# revision 15
# speedup vs baseline: 1.1367x; 1.1367x over previous
"""DCCA 2D loss kernel for 8 Trainium2 NeuronCores (Bass/Tile).

Strategy (data-parallel over the m = B*C = 2048 sample axis):
  - Each core gets 256 samples of both views, host-stacked into one tensor
    xx [2, NS, 64, 128].  Per chunk (16..64 samples) ONE SWDGE dma_start
    loads both views with f32->bf16 cast; partition = (sample-in-block)*8
    + n//8 so every descriptor reads a contiguous 4KB row-run.  Per
    16-sample block the PE transposes eight [128,128] slices (matmul
    against the identity), a vector/scalar copy assembles per-sample fused
    tiles T_m = [H1_m^T | H2_m^T] (n lands in a fixed even permutation,
    under which the final scalar is invariant), and the fused Gram
    G += T_m^T T_m accumulates in PSUM.  The ridge enters as the first
    PSUM matmul: weye^T weye = (R/(8*C1)) * I, so the AllReduce output is
    exactly S/C1 -- and the trace-normalized epilogue below is exactly
    invariant to the global scale C1, which therefore never appears.
  - One 64KB AllReduce(add) of the partial Gram (warmed up by a tiny
    AllReduce issued at t=0 so the ncfw control plane is hot).
  - Replicated epilogue without eigh: with  At = blockdiag(S11, S22)/C1,
    t = 128/tr(At), the closed-form first Newton-Schulz iterate
    X1 = t*(2I - t*At)  already inverts At to ~1e-4 (eigenvalues of t*At
    lie in [0.99, 1.01] for covariance data), optionally refined by
    BASS_NEWTON_ITERS more iterations (default 0).  Then
    corr^2 = sum( (X1 St)[0:64,64:128] * (St X1)[0:64,64:128] )  and
    out = -sqrt(corr^2).
"""

import math
import os

import numpy as np

# ---------------------------------------------------------------- constants
B, C, N, K = 32, 64, 64, 128
M = B * C                    # 2048 samples
NC = 8                       # cores
NS = M // NC                 # 256 samples per core
# Samples per DMA chunk. First chunk small so the PE starts within ~2us;
# tail tapered so the PE isn't left with a big block after the last byte.
CHUNKS = (16, 48, 64, 64, 32, 16, 16)
assert sum(CHUNKS) == NS
R_RIDGE = 1e-4
C1 = (1.0 - 1.0 / M) ** 2 / (M * (M - 1))
W_RIDGE = math.sqrt(R_RIDGE / (NC * C1))  # per-core PSUM ridge weight

NEWTON_ITERS = int(os.environ.get("BASS_NEWTON_ITERS", "0"))
G = 16                       # samples per partition-block
NR = 8                       # consecutive n-rows per descriptor (4KB)
NU = N // NR

_CACHE = {}


def _build():
    import contextlib

    import concourse.bass as bass
    import concourse.mybir as mybir
    import concourse.tile as tile
    from concourse import bacc

    f32 = mybir.dt.float32
    bf16 = mybir.dt.bfloat16

    nc = bacc.Bacc(
        "TRN2",
        target_bir_lowering=False,
        debug=False,
        enable_asserts=False,
        num_devices=NC,
    )

    xx = nc.dram_tensor(
        "xx", [NS // G, 2, G, N, K], f32, kind="ExternalInput"
    ).ap()
    ident_d = nc.dram_tensor("ident", [128, 128], bf16, kind="ExternalInput").ap()
    weye_d = nc.dram_tensor("weye", [128, 128], bf16, kind="ExternalInput").ap()
    eye2_d = nc.dram_tensor("eye2", [128, 128], f32, kind="ExternalInput").ap()
    maskd_d = nc.dram_tensor("maskd", [128, 128], f32, kind="ExternalInput").ap()
    ones_d = nc.dram_tensor("onesf", [128, 128], f32, kind="ExternalInput").ap()
    out_d = nc.dram_tensor("out", [1, 1], f32, kind="ExternalOutput").ap()

    groups = [list(range(NC))]

    with tile.TileContext(nc) as tc:
        with contextlib.ExitStack() as ctx:
            cpool = ctx.enter_context(tc.tile_pool(name="consts", bufs=1))
            spool = ctx.enter_context(tc.tile_pool(name="work", bufs=2))
            dpool = ctx.enter_context(tc.tile_pool(name="dram", bufs=1, space="DRAM"))

            # Dependency-free tiny collective issued before everything else:
            # wakes the ncfw control plane (first collective on this runtime
            # costs ~25us) so the real AllReduce later is warm.
            win = dpool.tile([1, 16], f32)
            wout = dpool.tile([NC, 16], f32)
            wsb = spool.tile([1, 16], f32, tag="wsb")
            nc.gpsimd.memset(wsb[:], 0.0)
            nc.gpsimd.dma_start(win[:], wsb[:])
            # Warmup kind MUST match the main collective: ncfw inits a
            # channel per collective type, and a cold channel costs ~20us.
            nc.gpsimd.collective_compute(
                "AllGather",
                mybir.AluOpType.bypass,
                replica_groups=groups,
                ins=[win.opt()],
                outs=[wout.opt()],
            )

            ident = cpool.tile([128, 128], bf16)
            nc.sync.dma_start(ident[:], ident_d)
            weye = cpool.tile([128, 128], bf16)
            nc.sync.dma_start(weye[:], weye_d)
            eye2 = cpool.tile([128, 128], f32)
            nc.sync.dma_start(eye2[:], eye2_d)
            maskd = cpool.tile([128, 128], f32)
            nc.sync.dma_start(maskd[:], maskd_d)
            onesf = cpool.tile([128, 128], f32)
            nc.sync.dma_start(onesf[:], ones_d)

            # Warm the SQRT activation table while Scalar is idle, so the
            # epilogue's sqrt doesn't pay ACT_TABLE_LOAD + drain (~2.5us).
            wsq = spool.tile([1, 1], f32, tag="wsq")
            nc.scalar.sqrt(wsq[:], onesf[0:1, 0:1])

            gsb = spool.tile([128, 128], f32, tag="gsb")

            # ---------------- main loop: per-core partial fused Gram ------
            # Per chunk/SBUF tile V [128, 2, CH/G, NR*128] (bf16):
            #   V[8h+u, v, j, r*128+k] = X[v, s0+G*j+h, u*NR+r, k]
            # Per G-sample block: 8 transposes (2 views x 4 r) as regular
            # matmuls against the identity (FWL-friendly), one assembly copy
            # per view into the fused TT tile, then G Gram matmuls.
            with (
                tc.tile_pool(name="vload", bufs=3) as vpool,
                tc.tile_pool(name="ttp", bufs=3) as ttpool,
                tc.tile_pool(name="ptp", bufs=2, space="PSUM") as ptpool,
                tc.tile_pool(name="gpp", bufs=1, space="PSUM") as gpool,
            ):
                gp = gpool.tile([128, 128], f32, tag="gp")
                # ridge: weye^T weye = (R/(8*C1)) I seeds the accumulator
                nc.tensor.matmul(gp[:], weye[:], weye[:], start=True, stop=False)

                n_blocks_total = NS // G

                def emit_gram(tt8, bi):
                    last = bi == n_blocks_total - 1
                    tt8f = tt8.rearrange("p h b u -> p (h b u)")
                    for h in range(G):
                        nc.tensor.matmul(
                            gp[:],
                            tt8f[:, 128 * h : 128 * (h + 1)],
                            tt8f[:, 128 * h : 128 * (h + 1)],
                            start=False,
                            stop=last and h == G - 1,
                        )

                pending = None  # (tt, block_index) awaiting Gram matmuls
                bi = 0
                s0 = 0
                for CH in CHUNKS:
                    nj = CH // G
                    vt = vpool.tile([128, 2 * nj, NR * 128], bf16, tag="v")
                    src = xx[s0 : s0 + nj].rearrange(
                        "j v h (u r) k -> (h u) (j v) (r k)", r=NR
                    )
                    # SWDGE casts f32 -> bf16 during the transfer.
                    nc.gpsimd.dma_start(vt[:], src)
                    s0 += nj

                    for j in range(nj):
                        tt = ttpool.tile([128, G, 2, 64], bf16, tag="tt")
                        for vi in range(2):
                            # bf16 PSUM transpose tile: values are already
                            # bf16-exact, and the PSUM->SBUF copy runs at the
                            # DVE 16-bit 2x rate.
                            ptv = ptpool.tile([128, NR, 128], bf16, tag=f"pt{vi}")
                            for r in range(NR):
                                nc.tensor.transpose(
                                    ptv[:, r, :],
                                    vt[:, 2 * j + vi, r * 128 : (r + 1) * 128],
                                    ident[:],
                                )
                            nc.any.tensor_copy(
                                out=tt[:, :, vi, :].rearrange(
                                    "p h (r u) -> p h r u", r=NR
                                ),
                                in_=ptv.rearrange("p r (h u) -> p h r u", h=G),
                            )
                        # one-block software pipeline: this block's Gram
                        # matmuls are emitted after the NEXT block's
                        # transposes, so the PE never stalls on the copy.
                        if pending is not None:
                            emit_gram(*pending)
                        pending = (tt, bi)
                        bi += 1
                emit_gram(*pending)

                nc.vector.tensor_copy(gsb[:], gp[:])

            # ---------------- AllReduce + replicated epilogue -------------
            with tc.tile_pool(name="epp", bufs=1, space="PSUM") as epool:
                din = dpool.tile([128, 128], f32)
                dout = dpool.tile([NC, 128, 128], f32)
                # HWDGE on the sync queue: skips the SWDGE queue tail.
                nc.sync.dma_start(din[:], gsb[:])
                # AllGather (one ncfw phase) + on-chip tree sum is ~8us
                # faster end-to-end than ncfw AllReduce (= RS + AG).
                nc.gpsimd.collective_compute(
                    "AllGather",
                    mybir.AluOpType.bypass,
                    replica_groups=groups,
                    ins=[din.opt()],
                    outs=[dout.opt()],
                )
                gall = spool.tile([128, NC, 128], f32, tag="gall")
                # split across both HWDGE queues (sync + scalar)
                nc.sync.dma_start(
                    gall[:, 0:4, :], dout[0:4].rearrange("c p k -> p c k")
                )
                nc.scalar.dma_start(
                    gall[:, 4:8, :], dout[4:8].rearrange("c p k -> p c k")
                )
                g4 = spool.tile([128, 4, 128], f32, tag="g4")
                nc.vector.tensor_add(g4[:], gall[:, 0:4, :], gall[:, 4:8, :])
                g2 = spool.tile([128, 2, 128], f32, tag="g2")
                nc.vector.tensor_add(g2[:], g4[:, 0:2, :], g4[:, 2:4, :])
                S = spool.tile([128, 128], f32, tag="S")
                nc.vector.tensor_add(S[:], g2[:, 0, :], g2[:, 1, :])

                # A = blockdiag(S); dm = 2*diag(S) (independent -> both DVE)
                A = spool.tile([128, 128], f32, tag="A")
                nc.vector.tensor_mul(A[:], S[:], maskd[:])
                dm = spool.tile([128, 128], f32, tag="dm")
                nc.vector.tensor_mul(dm[:], S[:], eye2[:])
                dcol = spool.tile([128, 1], f32, tag="dcol")
                nc.vector.reduce_sum(dcol[:], dm[:], axis=mybir.AxisListType.X)
                trp = epool.tile([128, 1], f32, tag="trp")
                nc.tensor.matmul(trp[:], onesf[:], dcol[:], start=True, stop=True)
                rcol = spool.tile([128, 1], f32, tag="rcol")
                nc.vector.reciprocal(rcol[:], trp[:])  # 1/(2 tr A)

                # Ahat = t*A (t = 128/trA = 256*rcol); Xhat = 2I - Ahat
                ahat = spool.tile([128, 128], f32, tag="ahat")
                nc.vector.tensor_scalar(
                    ahat[:], A[:], rcol[:], 256.0,
                    op0=mybir.AluOpType.mult, op1=mybir.AluOpType.mult,
                )
                xhat = spool.tile([128, 128], f32, tag="xh")
                nc.vector.tensor_tensor(
                    xhat[:], eye2[:], ahat[:], mybir.AluOpType.subtract
                )
                for _ in range(NEWTON_ITERS):
                    bp = epool.tile([128, 128], f32, tag="bp")
                    nc.tensor.matmul(bp[:], ahat[:], xhat[:], start=True, stop=True)
                    cs = spool.tile([128, 128], f32, tag="cs")
                    nc.vector.tensor_tensor(
                        cs[:], eye2[:], bp[:], mybir.AluOpType.subtract
                    )
                    xp = epool.tile([128, 128], f32, tag="xp")
                    nc.tensor.matmul(xp[:], xhat[:], cs[:], start=True, stop=True)
                    xnew = spool.tile([128, 128], f32, tag="xh")
                    nc.vector.tensor_copy(xnew[:], xp[:])
                    xhat = xnew
                # X1 = t * Xhat  (so X1 ~= inv(blockdiag) up to the global C1)
                x1 = spool.tile([128, 128], f32, tag="x1")
                nc.vector.tensor_scalar(
                    x1[:], xhat[:], rcol[:], 256.0,
                    op0=mybir.AluOpType.mult, op1=mybir.AluOpType.mult,
                )

                # corr^2 = sum( (X1 S)[0:64,64:] * (S X1)[0:64,64:] )
                up = epool.tile([64, 64], f32, tag="up")
                nc.tensor.matmul(up[:], x1[:, 0:64], S[:, 64:128], start=True, stop=True)
                vp = epool.tile([64, 64], f32, tag="vp")
                nc.tensor.matmul(vp[:], S[:, 0:64], x1[:, 64:128], start=True, stop=True)
                vps = spool.tile([64, 64], f32, tag="vps")
                nc.vector.tensor_copy(vps[:], vp[:])
                pm = spool.tile([64, 64], f32, tag="pm")
                nc.vector.tensor_tensor(pm[:], up[:], vps[:], mybir.AluOpType.mult)
                pcol = spool.tile([64, 1], f32, tag="pcol")
                nc.vector.reduce_sum(pcol[:], pm[:], axis=mybir.AxisListType.X)
                cp = epool.tile([1, 1], f32, tag="cp")
                nc.tensor.matmul(cp[:], pcol[:], onesf[0:64, 0:1], start=True, stop=True)
                root = spool.tile([1, 1], f32, tag="root")
                nc.scalar.sqrt(root[:], cp[:])
                nc.vector.tensor_scalar_mul(root[:], root[:], -1.0)
                nc.sync.dma_start(out_d, root[:])

    nc.compile()
    return nc


def _get_nc():
    key = (NEWTON_ITERS,)
    if key not in _CACHE:
        _CACHE[key] = _build()
    return _CACHE[key]


def _const_inputs():
    import ml_dtypes

    eye = np.eye(128, dtype=np.float32)
    maskd = np.zeros((128, 128), dtype=np.float32)
    maskd[:64, :64] = np.eye(64, dtype=np.float32)
    maskd[64:, 64:] = np.eye(64, dtype=np.float32)
    return {
        "ident": eye.astype(ml_dtypes.bfloat16),
        "weye": (W_RIDGE * eye).astype(ml_dtypes.bfloat16),
        "eye2": (2.0 * eye).astype(np.float32),
        "maskd": maskd,
        "onesf": np.ones((128, 128), dtype=np.float32),
    }


def kernel(data_view1, data_view2):
    from concourse import bass_utils

    h1 = np.ascontiguousarray(data_view1, dtype=np.float32).reshape(M, N, K)
    h2 = np.ascontiguousarray(data_view2, dtype=np.float32).reshape(M, N, K)

    consts = _const_inputs()
    in_maps = []
    for c in range(NC):
        m = {
            "xx": np.stack(
                [
                    h1[c * NS : (c + 1) * NS].reshape(NS // G, G, N, K),
                    h2[c * NS : (c + 1) * NS].reshape(NS // G, G, N, K),
                ],
                axis=1,
            ),
        }
        m.update(consts)
        in_maps.append(m)

    nc = _get_nc()
    trace = os.environ.get("BASS_KERNEL_TRACE", "0") == "1"
    res = bass_utils.run_bass_kernel_spmd(
        nc, in_maps, core_ids=list(range(NC)), trace=trace
    )
    if trace:
        kernel.last_results = res
    val = np.asarray(res.results[0]["out"]).reshape(())
    return val.astype(np.float32)
